# revision 9
# baseline (speedup 1.0000x reference)
"""DeepFwFM (nn_DeepFwFM_12610023981507) Bass/Tile kernel for 8 TRN2 cores.

Self-contained: accepts the FULL unsharded inputs, shards the batch across 8
NeuronCores (data parallel; weights replicated), runs one SPMD Bass kernel,
and gathers the per-core outputs into the full [16384] result.

Math (per sample b):
  V[b, f, d]: 13 numeric fields (num_emb * Xv) + 26 gathered cat rows.
  h = V.reshape(390)          (f-major: col = f*10 + d)
  first  = h . w                      (w = fwfm_w.flatten())
  second = 0.5 * h^T M h              (M = kron(S_offdiag, I10), S = 0.5(C+C^T))
  deep   = W3 @ relu(W2 @ relu(W1 h + b1) + b2)
  y = first + second + deep + bias

Device-side per 512-sample block: DMA Xi/Xv, DVE adds vocab offsets, POOL
indirect-DMA gathers the categorical embedding rows, DVE fills numeric
columns, PE transposes sample-major tiles into feature-major hT chunks, then
chunked PE matmuls compute U = M h, first, the MLP, and a ones-matmul
partition-reduce of E = U*h. ACT fuses relu+bias into the PSUM->SBUF copies.
K tail chunks (390 -> 6 rows, 400 -> 16 rows) use host-zero-padded weight
chunks and persistently zeroed rhs tail tiles, so every matmul runs K=128.
"""

import sys

for _p in ("/opt/trn_rl_repo", "/root/.axon_site/_ro/trn_rl_repo"):
    if _p not in sys.path:
        sys.path.append(_p)

import numpy as np

import concourse.bass as bass
import concourse.mybir as mybir
import concourse.tile as tile
from concourse import bacc
from concourse.bass_utils import run_bass_kernel_spmd
from concourse.masks import make_identity

P = 128
EMB = 10
NF = 39
NUM = 13
NCAT = 26
D_IN = NF * EMB  # 390
H1 = 400
H2 = 400
KC_IN = 4
KC_H = 4
R_TAIL_IN = D_IN - 3 * P   # 6
R_TAIL_H = H1 - 3 * P      # 16
F32 = mybir.dt.float32
I32 = mybir.dt.int32
N_CORES = 8


def _rows_in(c):
    return P if c < 3 else R_TAIL_IN


def _rows_h(c):
    return P if c < 3 else R_TAIL_H


def host_prepare(inputs, n_cores):
    """Shard the batch and pre-pack weights (reshape/pad/replicate only)."""
    Xi = np.asarray(inputs["Xi"])
    Xv = np.ascontiguousarray(np.asarray(inputs["Xv"], dtype=np.float32))
    cat_table = np.ascontiguousarray(np.asarray(inputs["cat_table"], dtype=np.float32))
    num_emb = np.asarray(inputs["num_emb"], dtype=np.float32)
    offsets = np.asarray(inputs["offsets"], dtype=np.int64)
    fwfm_w = np.asarray(inputs["fwfm_w"], dtype=np.float32)
    field_cov_w = np.asarray(inputs["field_cov_w"], dtype=np.float32)
    W1 = np.asarray(inputs["W1"], dtype=np.float32)
    b1 = np.asarray(inputs["b1"], dtype=np.float32)
    W2 = np.asarray(inputs["W2"], dtype=np.float32)
    b2 = np.asarray(inputs["b2"], dtype=np.float32)
    W3 = np.asarray(inputs["W3"], dtype=np.float32)
    bias = np.asarray(inputs["bias"], dtype=np.float32)

    B = Xi.shape[0]
    assert B % n_cores == 0
    Bc = B // n_cores

    S = 0.5 * (field_cov_w + field_cov_w.T)
    Soff = S.copy()
    np.fill_diagonal(Soff, 0.0)
    M = np.kron(Soff, np.eye(EMB, dtype=np.float32)).astype(np.float32)

    def kchunk(mat_t, kdim):
        kc = (kdim + P - 1) // P
        out = np.zeros((P, kc, mat_t.shape[1]), dtype=np.float32)
        for c in range(kc):
            r = min(P, kdim - c * P)
            out[:r, c, :] = mat_t[c * P:c * P + r, :]
        return out

    T_UF = kchunk(M, D_IN)
    W1T = kchunk(W1.T.copy(), D_IN)
    W2T = kchunk(W2.T.copy(), H1)
    w3 = kchunk(W3.T.copy(), H2)
    wfirst = kchunk(fwfm_w.reshape(D_IN, 1), D_IN)
    ones_pad = np.zeros((P, KC_IN, 1), dtype=np.float32)
    ones_pad[:, :3, 0] = 1.0
    ones_pad[:R_TAIL_IN, 3, 0] = 1.0

    def mchunk_vec(v, mdim):
        mc = (mdim + P - 1) // P
        out = np.zeros((P, mc), dtype=np.float32)
        for c in range(mc):
            r = min(P, mdim - c * P)
            out[:r, c] = v[c * P:c * P + r]
        return out

    b1_sb = mchunk_vec(b1, H1)
    b2_sb = mchunk_vec(b2, H2)
    num_bcast = np.broadcast_to(
        num_emb.reshape(1, NUM * EMB), (P, NUM * EMB)
    ).copy().astype(np.float32)
    bias_sb = bias.reshape(1, 1).astype(np.float32)

    Xi2 = np.ascontiguousarray(Xi.reshape(B, NCAT).astype(np.int32))

    # flat byte view of the table: indirect DMA consumes byte offsets
    # (coef=1) which the SWDGE turns into descriptors ~2.4x faster than
    # row-index mode (HW-measured 206 vs 493 ns/row).
    cat_u8 = cat_table.reshape(-1).view(np.uint8).reshape(-1, 1)

    shared = dict(
        cat_table=cat_u8, T_UF=T_UF, W1T=W1T, W2T=W2T, w3=w3,
        wfirst=wfirst, ones_pad=ones_pad, b1_sb=b1_sb, b2_sb=b2_sb,
        num_bcast=num_bcast, bias_sb=bias_sb,
    )
    in_maps = []
    for c in range(n_cores):
        m = dict(shared)
        m["Xi"] = Xi2[c * Bc:(c + 1) * Bc]
        m["Xv"] = np.ascontiguousarray(Xv[c * Bc:(c + 1) * Bc])
        in_maps.append(m)
    return in_maps, [int(v) for v in offsets]


def emit_dfm(tc, outs, ins, Bc, field_offsets, gather_splits=2, dbg=False,
             repeat=1):
    """Emit the per-core kernel IR. outs/ins are dicts of DRAM APs."""
    nc = tc.nc
    SB = 512
    assert Bc % SB == 0
    n_blocks = Bc // SB
    KK = SB // P

    Xi = ins["Xi"]
    Xv = ins["Xv"]
    ct = ins["cat_table"]
    y = outs["y"]

    import contextlib
    ctx = contextlib.ExitStack()
    with ctx:
        const = ctx.enter_context(tc.tile_pool(name="const", bufs=1))
        data = ctx.enter_context(tc.tile_pool(name="data", bufs=2))
        hpool = ctx.enter_context(tc.tile_pool(name="hpool", bufs=6))
        epool = ctx.enter_context(tc.tile_pool(name="epool", bufs=2))
        ypool = ctx.enter_context(tc.tile_pool(name="ypool", bufs=2))
        pt = ctx.enter_context(tc.tile_pool(name="pt", bufs=2, space="PSUM"))
        pmm = ctx.enter_context(tc.tile_pool(name="pmm", bufs=4, space="PSUM"))
        psm = ctx.enter_context(tc.tile_pool(name="psm", bufs=2, space="PSUM"))

        def load_const(name, shape, dtype=F32):
            t = const.tile(list(shape), dtype, tag=name)
            nc.sync.dma_start(t[:], ins[name][:])
            return t

        T_UF = load_const("T_UF", [P, KC_IN, D_IN])
        W1T = load_const("W1T", [P, KC_IN, H1])
        W2T = load_const("W2T", [P, KC_H, H2])
        w3 = load_const("w3", [P, KC_H, 1])
        wfirst = load_const("wfirst", [P, KC_IN, 1])
        ones_sb = load_const("ones_pad", [P, KC_IN, 1])
        b1_sb = load_const("b1_sb", [P, KC_H])
        b2_sb = load_const("b2_sb", [P, KC_H])
        numb = load_const("num_bcast", [P, NUM * EMB])
        bias_sb = load_const("bias_sb", [1, 1])

        ident = const.tile([P, P], F32, tag="ident")
        make_identity(nc, ident[:])

        hT3 = const.tile([P, SB], F32, tag="hT3")
        E3 = const.tile([P, SB], F32, tag="E3")
        h1t = const.tile([P, SB], F32, tag="h1t")
        h2t = const.tile([P, SB], F32, tag="h2t")
        for t in (hT3, E3, h1t, h2t):
            nc.vector.memset(t[:], 0.0)

        for bl in [b for _ in range(repeat) for b in range(n_blocks)]:
            s0 = bl * SB
            xi_sb = data.tile([P, KK, NCAT], I32, tag="xi")
            nc.sync.dma_start(
                xi_sb[:], Xi[s0:s0 + SB, :].rearrange("(k p) f -> p k f", p=P)
            )
            xv_sb = data.tile([P, KK, NUM], F32, tag="xv")
            nc.sync.dma_start(
                xv_sb[:], Xv[s0:s0 + SB, :NUM].rearrange("(k p) f -> p k f", p=P)
            )
            # local byte offset = (Xi*5)*8. Elementwise int32 on DVE/POOL
            # is fp32-backed (exact only when the mantissa fits 24 bits):
            # Xi*5 <= 10M is exact, and *8 scales the exponent only; the
            # per-field table base goes in via the static element_offset.
            nc.vector.tensor_scalar(
                out=xi_sb[:], in0=xi_sb[:], scalar1=5, scalar2=8,
                op0=mybir.AluOpType.mult,
                op1=mybir.AluOpType.mult,
            )

            V_sb = data.tile([P, KK, D_IN], F32, tag="V")
            # one 128-row gather per (sample-tile, field): HW honors one
            # index per partition per indirect DMA.
            for kk in range(KK):
                for f in range(NCAT):
                    c0 = NUM * EMB + f * EMB
                    nc.gpsimd.indirect_dma_start(
                        out=V_sb[:, kk, c0:c0 + EMB].bitcast(mybir.dt.uint8),
                        out_offset=None,
                        in_=ct[:, :],
                        in_offset=bass.IndirectOffsetOnAxis(
                            ap=xi_sb[:, kk, f:f + 1], axis=0
                        ),
                        element_offset=field_offsets[f] * EMB * 4,
                    )
            nc.vector.tensor_tensor(
                out=V_sb[:, :, :NUM * EMB].rearrange(
                    "p k (f d) -> p k f d", d=EMB
                ),
                in0=numb[:].rearrange("p (f d) -> p f d", d=EMB)[
                    :, None
                ].to_broadcast([P, KK, NUM, EMB]),
                in1=xv_sb[:, :, :, None].to_broadcast([P, KK, NUM, EMB]),
                op=mybir.AluOpType.mult,
            )

            hT = []
            for c in range(KC_IN):
                r = _rows_in(c)
                dst = hT3 if c == 3 else hpool.tile([P, SB], F32, tag="hT")
                for kk in range(KK):
                    ps = pt.tile([P, P], F32, tag="pt")
                    nc.tensor.transpose(
                        ps[:r, :], V_sb[:, kk, c * P:c * P + r], ident[:]
                    )
                    nc.vector.tensor_copy(
                        out=dst[:r, kk * P:(kk + 1) * P], in_=ps[:r, :]
                    )
                hT.append(dst)

            Es = psm.tile([1, SB], F32, tag="small")
            for m in range(KC_IN):
                r = _rows_in(m)
                ups = pmm.tile([P, SB], F32, tag="mm")
                for k in range(KC_IN):
                    nc.tensor.matmul(
                        ups[:r, :],
                        lhsT=T_UF[:, k, m * P:m * P + r],
                        rhs=hT[k][:, :],
                        start=(k == 0),
                        stop=(k == KC_IN - 1),
                    )
                Em = E3 if m == 3 else epool.tile([P, SB], F32, tag="E")
                nc.vector.tensor_tensor(
                    out=Em[:r, :], in0=ups[:r, :], in1=hT[m][:r, :],
                    op=mybir.AluOpType.mult,
                )
                nc.tensor.matmul(
                    Es[:, :], lhsT=ones_sb[:, m, :], rhs=Em[:, :],
                    start=(m == 0), stop=(m == KC_IN - 1),
                )
            Fs = psm.tile([1, SB], F32, tag="small")
            for k in range(KC_IN):
                nc.tensor.matmul(
                    Fs[:, :], lhsT=wfirst[:, k, :], rhs=hT[k][:, :],
                    start=(k == 0), stop=(k == KC_IN - 1),
                )

            h1 = []
            for m in range(KC_H):
                r = _rows_h(m)
                ps = pmm.tile([P, SB], F32, tag="mm")
                for k in range(KC_IN):
                    nc.tensor.matmul(
                        ps[:r, :],
                        lhsT=W1T[:, k, m * P:m * P + r],
                        rhs=hT[k][:, :],
                        start=(k == 0),
                        stop=(k == KC_IN - 1),
                    )
                dst = h1t if m == 3 else hpool.tile([P, SB], F32, tag="h1")
                nc.scalar.activation(
                    dst[:r, :], ps[:r, :],
                    mybir.ActivationFunctionType.Relu,
                    bias=b1_sb[:r, m:m + 1],
                )
                h1.append(dst)

            h2 = []
            for m in range(KC_H):
                r = _rows_h(m)
                ps = pmm.tile([P, SB], F32, tag="mm")
                for k in range(KC_H):
                    nc.tensor.matmul(
                        ps[:r, :],
                        lhsT=W2T[:, k, m * P:m * P + r],
                        rhs=h1[k][:, :],
                        start=(k == 0),
                        stop=(k == KC_H - 1),
                    )
                dst = h2t if m == 3 else hpool.tile([P, SB], F32, tag="h2")
                nc.scalar.activation(
                    dst[:r, :], ps[:r, :],
                    mybir.ActivationFunctionType.Relu,
                    bias=b2_sb[:r, m:m + 1],
                )
                h2.append(dst)

            Ds = psm.tile([1, SB], F32, tag="small")
            for k in range(KC_H):
                nc.tensor.matmul(
                    Ds[:, :], lhsT=w3[:, k, :], rhs=h2[k][:, :],
                    start=(k == 0), stop=(k == KC_H - 1),
                )

            y_sb = ypool.tile([1, SB], F32, tag="y")
            nc.vector.tensor_scalar(
                out=y_sb[:], in0=Es[:], scalar1=0.5, scalar2=None,
                op0=mybir.AluOpType.mult,
            )
            nc.vector.tensor_tensor(
                out=y_sb[:], in0=y_sb[:], in1=Fs[:], op=mybir.AluOpType.add
            )
            nc.vector.tensor_tensor(
                out=y_sb[:], in0=y_sb[:], in1=Ds[:], op=mybir.AluOpType.add
            )
            nc.vector.tensor_scalar(
                out=y_sb[:], in0=y_sb[:], scalar1=bias_sb[:1, :1], scalar2=None,
                op0=mybir.AluOpType.add,
            )
            nc.sync.dma_start(out=y[None, s0:s0 + SB], in_=y_sb[:])
            if dbg:
                nc.sync.dma_start(
                    out=outs["dbg_V"][s0:s0 + SB, :].rearrange(
                        "(k p) d -> p k d", p=P),
                    in_=V_sb[:],
                )
                for nm, ps in (("dbg_first", Fs), ("dbg_second", Es),
                               ("dbg_deep", Ds)):
                    t = ypool.tile([1, SB], F32, tag=nm)
                    nc.vector.tensor_copy(out=t[:], in_=ps[:])
                    nc.sync.dma_start(out=outs[nm][None, s0:s0 + SB], in_=t[:])


def _build_module(in_map, Bc, field_offsets, gather_splits=2, dbg=False,
                  repeat=1):
    nc = bacc.Bacc(None, target_bir_lowering=False, debug=False,
                   num_devices=N_CORES)
    ins = {}
    dt_map = {np.dtype(np.int32): I32, np.dtype(np.float32): F32,
              np.dtype(np.uint8): mybir.dt.uint8}
    for name, arr in in_map.items():
        ins[name] = nc.dram_tensor(
            name, list(arr.shape), dt_map[arr.dtype], kind="ExternalInput"
        ).ap()
    outs = {"y": nc.dram_tensor("y", [Bc], F32, kind="ExternalOutput").ap()}
    if dbg:
        outs["dbg_V"] = nc.dram_tensor("dbg_V", [Bc, D_IN], F32,
                                       kind="ExternalOutput").ap()
        for nm in ("dbg_first", "dbg_second", "dbg_deep"):
            outs[nm] = nc.dram_tensor(nm, [Bc], F32, kind="ExternalOutput").ap()
    with tile.TileContext(nc) as tc:
        emit_dfm(tc, outs, ins, Bc=Bc, field_offsets=field_offsets,
                 gather_splits=gather_splits, dbg=dbg, repeat=repeat)
    nc.compile()
    return nc


def run(inputs, trace=False, gather_splits=2, dbg=False, n_cores=None,
        **run_kwargs):
    """Run on 8 cores; returns (y_full, BassKernelResults)."""
    ncores = n_cores or N_CORES
    in_maps, field_offsets = host_prepare(inputs, ncores)
    Bc = in_maps[0]["Xi"].shape[0]
    nc = _build_module(in_maps[0], Bc, field_offsets,
                       gather_splits=gather_splits, dbg=dbg)
    res = run_bass_kernel_spmd(
        nc, in_maps, core_ids=list(range(ncores)), trace=trace, **run_kwargs
    )
    y = np.concatenate([r["y"].reshape(-1) for r in res.results])
    return y.astype(np.float32), res


def kernel(**inputs):
    y, _ = run(inputs, trace=False)
    return y


# revision 14
# speedup vs baseline: 1.1285x; 1.1285x over previous
"""DeepFwFM (nn_DeepFwFM_12610023981507) Bass/Tile kernel for 8 TRN2 cores.

Self-contained: accepts the FULL unsharded inputs, shards the batch across 8
NeuronCores (data parallel; weights replicated), runs one SPMD Bass kernel,
and gathers the per-core outputs into the full [16384] result.

Math (per sample b):
  V[b, f, d]: 13 numeric fields (num_emb * Xv) + 26 gathered cat rows.
  h = V.reshape(390)          (f-major: col = f*10 + d)
  first  = h . w                      (w = fwfm_w.flatten())
  second = 0.5 * h^T M h              (M = kron(S_offdiag, I10), S = 0.5(C+C^T))
  deep   = W3 @ relu(W2 @ relu(W1 h + b1) + b2)
  y = first + second + deep + bias

Device-side per 512-sample block: DMA Xi/Xv, DVE adds vocab offsets, POOL
indirect-DMA gathers the categorical embedding rows, DVE fills numeric
columns, PE transposes sample-major tiles into feature-major hT chunks, then
chunked PE matmuls compute U = M h, first, the MLP, and a ones-matmul
partition-reduce of E = U*h. ACT fuses relu+bias into the PSUM->SBUF copies.
K tail chunks (390 -> 6 rows, 400 -> 16 rows) use host-zero-padded weight
chunks and persistently zeroed rhs tail tiles, so every matmul runs K=128.
"""

import sys

for _p in ("/opt/trn_rl_repo", "/root/.axon_site/_ro/trn_rl_repo"):
    if _p not in sys.path:
        sys.path.append(_p)

import numpy as np

import concourse.bass as bass
import concourse.mybir as mybir
import concourse.tile as tile
from concourse import bacc
from concourse.bass_utils import run_bass_kernel_spmd
from concourse.masks import make_identity

P = 128
EMB = 10
NF = 39
NUM = 13
NCAT = 26
D_IN = NF * EMB  # 390
H1 = 400
H2 = 400
KC_IN = 4
KC_H = 4
R_TAIL_IN = D_IN - 3 * P   # 6
R_TAIL_H = H1 - 3 * P      # 16
F32 = mybir.dt.float32
I32 = mybir.dt.int32
N_CORES = 8


def _rows_in(c):
    return P if c < 3 else R_TAIL_IN


def _rows_h(c):
    return P if c < 3 else R_TAIL_H


def host_prepare(inputs, n_cores):
    """Shard the batch; pack weights; build gather-side tables.

    Cat fields are permuted to [small-group-A | small-group-B | big]:
    smalls (vocab <= 32767) are gathered via two per-block dma_gathers from
    256B-row repacked side tables (int16 indices); bigs stay on per-field
    byte-offset indirect DMA. All weights are permuted to match.
    """
    Xi = np.asarray(inputs["Xi"])
    Xv = np.ascontiguousarray(np.asarray(inputs["Xv"], dtype=np.float32))
    cat_table = np.ascontiguousarray(np.asarray(inputs["cat_table"], dtype=np.float32))
    num_emb = np.asarray(inputs["num_emb"], dtype=np.float32)
    offsets = np.asarray(inputs["offsets"], dtype=np.int64)
    fwfm_w = np.asarray(inputs["fwfm_w"], dtype=np.float32)
    field_cov_w = np.asarray(inputs["field_cov_w"], dtype=np.float32)
    W1 = np.asarray(inputs["W1"], dtype=np.float32)
    b1 = np.asarray(inputs["b1"], dtype=np.float32)
    W2 = np.asarray(inputs["W2"], dtype=np.float32)
    b2 = np.asarray(inputs["b2"], dtype=np.float32)
    W3 = np.asarray(inputs["W3"], dtype=np.float32)
    bias = np.asarray(inputs["bias"], dtype=np.float32)

    B = Xi.shape[0]
    assert B % n_cores == 0
    Bc = B // n_cores
    Xi2 = np.ascontiguousarray(Xi.reshape(B, NCAT).astype(np.int64))

    # field sizes from offsets (sizes[i] = offsets[i+1]-offsets[i])
    total_rows = cat_table.shape[0]
    sizes = np.diff(np.concatenate([offsets, [total_rows]])).astype(np.int64)

    smalls = [f for f in range(NCAT) if sizes[f] <= 32767]
    bigs = [f for f in range(NCAT) if sizes[f] > 32767]
    groupA, groupB, accA, accB = [], [], 0, 0
    for f in smalls:
        if accA + sizes[f] <= 32768:
            groupA.append(f); accA += sizes[f]
        else:
            assert accB + sizes[f] <= 32768, "small fields exceed two groups"
            groupB.append(f); accB += sizes[f]
    cat_perm = groupA + groupB + bigs
    nA, nB, nBig = len(groupA), len(groupB), len(bigs)

    def packed_table(group):
        rows = int(sum(sizes[f] for f in group))
        t = np.zeros((rows, 64), dtype=np.float32)
        base, local_off = 0, {}
        for f in group:
            n = int(sizes[f])
            t[base:base + n, :EMB] = cat_table[offsets[f]:offsets[f] + n]
            local_off[f] = base
            base += n
        return t, local_off

    tabA, offA = packed_table(groupA)
    tabB, offB = packed_table(groupB)

    # per-block wrapped int16 indices for each group, all cores
    n_blocks_total = B // 512

    def wrapped_idx(group, loff):
        if not group:
            return np.zeros((n_blocks_total, 128, 0), np.int16)
        cols = np.stack(
            [Xi2[:, f] + loff[f] for f in group], axis=0
        ).astype(np.int16)                          # [nG, B]
        out = np.zeros((n_blocks_total, 16, len(group) * 32),
                       dtype=np.int16)
        for blk in range(n_blocks_total):
            flat = cols[:, blk * 512:(blk + 1) * 512].reshape(-1)  # f-major
            i = np.arange(len(flat))
            out[blk, i % 16, i // 16] = flat
        return np.tile(out, (1, 8, 1)).reshape(n_blocks_total, 128, -1)

    idxA = wrapped_idx(groupA, offA)
    idxB = wrapped_idx(groupB, offB)
    Xi_big = np.ascontiguousarray(Xi2[:, bigs].astype(np.int32))

    # ---- permuted weights (field order: numeric + cat_perm) ----
    perm39 = list(range(NUM)) + [NUM + f for f in cat_perm]
    S = 0.5 * (field_cov_w + field_cov_w.T)
    Soff = S.copy()
    np.fill_diagonal(Soff, 0.0)
    Sp = Soff[np.ix_(perm39, perm39)]
    M = np.kron(Sp, np.eye(EMB, dtype=np.float32)).astype(np.float32)
    W1p = W1.reshape(H1, NF, EMB)[:, perm39].reshape(H1, D_IN)
    fwp = fwfm_w[perm39].reshape(D_IN, 1)

    def kchunk(mat_t, kdim):
        kc = (kdim + P - 1) // P
        out = np.zeros((P, kc, mat_t.shape[1]), dtype=np.float32)
        for c in range(kc):
            r = min(P, kdim - c * P)
            out[:r, c, :] = mat_t[c * P:c * P + r, :]
        return out

    T_UF = kchunk(M, D_IN)
    W1T = kchunk(W1p.T.copy(), D_IN)
    W2T = kchunk(W2.T.copy(), H1)
    w3 = kchunk(W3.T.copy(), H2)
    wfirst = kchunk(fwp, D_IN)
    ones_pad = np.zeros((P, KC_IN, 1), dtype=np.float32)
    ones_pad[:, :3, 0] = 1.0
    ones_pad[:R_TAIL_IN, 3, 0] = 1.0

    def mchunk_vec(v, mdim):
        mc = (mdim + P - 1) // P
        out = np.zeros((P, mc), dtype=np.float32)
        for c in range(mc):
            r = min(P, mdim - c * P)
            out[:r, c] = v[c * P:c * P + r]
        return out

    b1_sb = mchunk_vec(b1, H1)
    b2_sb = mchunk_vec(b2, H2)
    num_bcast = np.broadcast_to(
        num_emb.reshape(1, NUM * EMB), (P, NUM * EMB)
    ).copy().astype(np.float32)
    bias_sb = bias.reshape(1, 1).astype(np.float32)

    cat_u8 = cat_table.reshape(-1).view(np.uint8).reshape(-1, 1)

    shared = dict(
        cat_table=cat_u8, tabA=tabA, tabB=tabB,
        T_UF=T_UF, W1T=W1T, W2T=W2T, w3=w3,
        wfirst=wfirst, ones_pad=ones_pad, b1_sb=b1_sb, b2_sb=b2_sb,
        num_bcast=num_bcast, bias_sb=bias_sb,
    )
    nb_core = Bc // 512
    in_maps = []
    for c in range(n_cores):
        m = dict(shared)
        m["Xi_big"] = Xi_big[c * Bc:(c + 1) * Bc]
        m["Xv"] = np.ascontiguousarray(Xv[c * Bc:(c + 1) * Bc])
        m["idxA"] = np.ascontiguousarray(
            idxA[c * nb_core:(c + 1) * nb_core].transpose(1, 0, 2))
        m["idxB"] = np.ascontiguousarray(
            idxB[c * nb_core:(c + 1) * nb_core].transpose(1, 0, 2))
        in_maps.append(m)
    meta = dict(nA=nA, nB=nB,
                big_offsets=[int(offsets[f]) for f in bigs])
    return in_maps, meta


def emit_dfm(tc, outs, ins, Bc, meta, gather_splits=2, dbg=False,
             repeat=1):
    """Emit the per-core kernel IR. outs/ins are dicts of DRAM APs."""
    nc = tc.nc
    SB = 512
    assert Bc % SB == 0
    n_blocks = Bc // SB
    KK = SB // P

    Xv = ins["Xv"]
    ct = ins["cat_table"]
    y = outs["y"]
    nA, nB = meta["nA"], meta["nB"]
    big_offsets = meta["big_offsets"]
    nBig = len(big_offsets)
    cA0 = NUM * EMB
    cB0 = cA0 + nA * EMB
    cG0 = cB0 + nB * EMB

    import contextlib
    ctx = contextlib.ExitStack()
    with ctx:
        const = ctx.enter_context(tc.tile_pool(name="const", bufs=1))
        data = ctx.enter_context(tc.tile_pool(name="data", bufs=2))
        hpool = ctx.enter_context(tc.tile_pool(name="hpool", bufs=6))
        epool = ctx.enter_context(tc.tile_pool(name="epool", bufs=2))
        ypool = ctx.enter_context(tc.tile_pool(name="ypool", bufs=2))
        pt = ctx.enter_context(tc.tile_pool(name="pt", bufs=2, space="PSUM"))
        pmm = ctx.enter_context(tc.tile_pool(name="pmm", bufs=4, space="PSUM"))
        psm = ctx.enter_context(tc.tile_pool(name="psm", bufs=2, space="PSUM"))

        def load_const(name, shape, dtype=F32):
            t = const.tile(list(shape), dtype, tag=name)
            nc.sync.dma_start(t[:], ins[name][:])
            return t

        T_UF = load_const("T_UF", [P, KC_IN, D_IN])
        W1T = load_const("W1T", [P, KC_IN, H1])
        W2T = load_const("W2T", [P, KC_H, H2])
        w3 = load_const("w3", [P, KC_H, 1])
        wfirst = load_const("wfirst", [P, KC_IN, 1])
        ones_sb = load_const("ones_pad", [P, KC_IN, 1])
        b1_sb = load_const("b1_sb", [P, KC_H])
        b2_sb = load_const("b2_sb", [P, KC_H])
        numb = load_const("num_bcast", [P, NUM * EMB])
        bias_sb = load_const("bias_sb", [1, 1])

        ident = const.tile([P, P], F32, tag="ident")
        make_identity(nc, ident[:])

        hT3 = const.tile([P, SB], F32, tag="hT3")
        E3 = const.tile([P, SB], F32, tag="E3")
        h1t = const.tile([P, SB], F32, tag="h1t")
        h2t = const.tile([P, SB], F32, tag="h2t")
        for t in (hT3, E3, h1t, h2t):
            nc.vector.memset(t[:], 0.0)

        for bl in [b for _ in range(repeat) for b in range(n_blocks)]:
            s0 = bl * SB
            if nBig:
                xi_sb = data.tile([P, KK, nBig], I32, tag="xi")
                nc.sync.dma_start(
                    xi_sb[:],
                    ins["Xi_big"][s0:s0 + SB, :].rearrange(
                        "(k p) f -> p k f", p=P),
                )
            xv_sb = data.tile([P, KK, NUM], F32, tag="xv")
            nc.sync.dma_start(
                xv_sb[:], Xv[s0:s0 + SB, :NUM].rearrange("(k p) f -> p k f", p=P)
            )
            # local byte offset = (Xi*5)*8: int32 elementwise is fp32-backed,
            # exact only while the mantissa fits 24 bits; per-field bases go
            # in via the gathers' static element_offset.
            if nBig:
                nc.vector.tensor_scalar(
                    out=xi_sb[:], in0=xi_sb[:], scalar1=5, scalar2=8,
                    op0=mybir.AluOpType.mult,
                    op1=mybir.AluOpType.mult,
                )

            V_sb = data.tile([P, KK, D_IN], F32, tag="V")

            # smalls: one dma_gather per group from 256B-row side tables
            for nm, nG, c0 in (("A", nA, cA0), ("B", nB, cB0)):
                if nG == 0:
                    continue
                ixt = data.tile([P, nG * 32], mybir.dt.int16, tag="ix" + nm)
                nc.sync.dma_start(ixt[:], ins["idx" + nm][:, bl, :])
                G = data.tile([P, KK * nG, 64], F32, tag="G" + nm)
                nc.gpsimd.dma_gather(
                    out_ap=G[:],
                    in_ap=ins["tab" + nm][:, :],
                    idxs_ap=ixt[:],
                    num_idxs=nG * SB,
                    num_idxs_reg=nG * SB,
                    elem_size=64,
                    single_packet=False,
                )
                nc.vector.tensor_copy(
                    out=V_sb[:, :, c0:c0 + nG * EMB].rearrange(
                        "p k (f d) -> p k f d", d=EMB),
                    in_=G[:, :, :EMB].rearrange(
                        "p (f k) d -> p k f d", k=KK),
                )

            # bigs: per-(subtile, field) byte-offset indirect gathers
            for kk in range(KK):
                for j in range(nBig):
                    c0 = cG0 + j * EMB
                    nc.gpsimd.indirect_dma_start(
                        out=V_sb[:, kk, c0:c0 + EMB].bitcast(mybir.dt.uint8),
                        out_offset=None,
                        in_=ct[:, :],
                        in_offset=bass.IndirectOffsetOnAxis(
                            ap=xi_sb[:, kk, j:j + 1], axis=0
                        ),
                        element_offset=big_offsets[j] * EMB * 4,
                    )
            nc.vector.tensor_tensor(
                out=V_sb[:, :, :NUM * EMB].rearrange(
                    "p k (f d) -> p k f d", d=EMB
                ),
                in0=numb[:].rearrange("p (f d) -> p f d", d=EMB)[
                    :, None
                ].to_broadcast([P, KK, NUM, EMB]),
                in1=xv_sb[:, :, :, None].to_broadcast([P, KK, NUM, EMB]),
                op=mybir.AluOpType.mult,
            )

            hT = []
            for c in range(KC_IN):
                r = _rows_in(c)
                dst = hT3 if c == 3 else hpool.tile([P, SB], F32, tag="hT")
                for kk in range(KK):
                    ps = pt.tile([P, P], F32, tag="pt")
                    nc.tensor.transpose(
                        ps[:r, :], V_sb[:, kk, c * P:c * P + r], ident[:]
                    )
                    nc.vector.tensor_copy(
                        out=dst[:r, kk * P:(kk + 1) * P], in_=ps[:r, :]
                    )
                hT.append(dst)

            Es = psm.tile([1, SB], F32, tag="small")
            for m in range(KC_IN):
                r = _rows_in(m)
                ups = pmm.tile([P, SB], F32, tag="mm")
                for k in range(KC_IN):
                    nc.tensor.matmul(
                        ups[:r, :],
                        lhsT=T_UF[:, k, m * P:m * P + r],
                        rhs=hT[k][:, :],
                        start=(k == 0),
                        stop=(k == KC_IN - 1),
                    )
                Em = E3 if m == 3 else epool.tile([P, SB], F32, tag="E")
                nc.vector.tensor_tensor(
                    out=Em[:r, :], in0=ups[:r, :], in1=hT[m][:r, :],
                    op=mybir.AluOpType.mult,
                )
                nc.tensor.matmul(
                    Es[:, :], lhsT=ones_sb[:, m, :], rhs=Em[:, :],
                    start=(m == 0), stop=(m == KC_IN - 1),
                )
            Fs = psm.tile([1, SB], F32, tag="small")
            for k in range(KC_IN):
                nc.tensor.matmul(
                    Fs[:, :], lhsT=wfirst[:, k, :], rhs=hT[k][:, :],
                    start=(k == 0), stop=(k == KC_IN - 1),
                )

            h1 = []
            for m in range(KC_H):
                r = _rows_h(m)
                ps = pmm.tile([P, SB], F32, tag="mm")
                for k in range(KC_IN):
                    nc.tensor.matmul(
                        ps[:r, :],
                        lhsT=W1T[:, k, m * P:m * P + r],
                        rhs=hT[k][:, :],
                        start=(k == 0),
                        stop=(k == KC_IN - 1),
                    )
                dst = h1t if m == 3 else hpool.tile([P, SB], F32, tag="h1")
                nc.scalar.activation(
                    dst[:r, :], ps[:r, :],
                    mybir.ActivationFunctionType.Relu,
                    bias=b1_sb[:r, m:m + 1],
                )
                h1.append(dst)

            h2 = []
            for m in range(KC_H):
                r = _rows_h(m)
                ps = pmm.tile([P, SB], F32, tag="mm")
                for k in range(KC_H):
                    nc.tensor.matmul(
                        ps[:r, :],
                        lhsT=W2T[:, k, m * P:m * P + r],
                        rhs=h1[k][:, :],
                        start=(k == 0),
                        stop=(k == KC_H - 1),
                    )
                dst = h2t if m == 3 else hpool.tile([P, SB], F32, tag="h2")
                nc.scalar.activation(
                    dst[:r, :], ps[:r, :],
                    mybir.ActivationFunctionType.Relu,
                    bias=b2_sb[:r, m:m + 1],
                )
                h2.append(dst)

            Ds = psm.tile([1, SB], F32, tag="small")
            for k in range(KC_H):
                nc.tensor.matmul(
                    Ds[:, :], lhsT=w3[:, k, :], rhs=h2[k][:, :],
                    start=(k == 0), stop=(k == KC_H - 1),
                )

            y_sb = ypool.tile([1, SB], F32, tag="y")
            nc.vector.tensor_scalar(
                out=y_sb[:], in0=Es[:], scalar1=0.5, scalar2=None,
                op0=mybir.AluOpType.mult,
            )
            nc.vector.tensor_tensor(
                out=y_sb[:], in0=y_sb[:], in1=Fs[:], op=mybir.AluOpType.add
            )
            nc.vector.tensor_tensor(
                out=y_sb[:], in0=y_sb[:], in1=Ds[:], op=mybir.AluOpType.add
            )
            nc.vector.tensor_scalar(
                out=y_sb[:], in0=y_sb[:], scalar1=bias_sb[:1, :1], scalar2=None,
                op0=mybir.AluOpType.add,
            )
            nc.sync.dma_start(out=y[None, s0:s0 + SB], in_=y_sb[:])
            if dbg:
                nc.sync.dma_start(
                    out=outs["dbg_V"][s0:s0 + SB, :].rearrange(
                        "(k p) d -> p k d", p=P),
                    in_=V_sb[:],
                )
                for nm, ps in (("dbg_first", Fs), ("dbg_second", Es),
                               ("dbg_deep", Ds)):
                    t = ypool.tile([1, SB], F32, tag=nm)
                    nc.vector.tensor_copy(out=t[:], in_=ps[:])
                    nc.sync.dma_start(out=outs[nm][None, s0:s0 + SB], in_=t[:])


def _build_module(in_map, Bc, meta, gather_splits=2, dbg=False,
                  repeat=1):
    nc = bacc.Bacc(None, target_bir_lowering=False, debug=False,
                   num_devices=N_CORES)
    ins = {}
    dt_map = {np.dtype(np.int32): I32, np.dtype(np.float32): F32,
              np.dtype(np.uint8): mybir.dt.uint8,
              np.dtype(np.int16): mybir.dt.int16}
    for name, arr in in_map.items():
        ins[name] = nc.dram_tensor(
            name, list(arr.shape), dt_map[arr.dtype], kind="ExternalInput"
        ).ap()
    outs = {"y": nc.dram_tensor("y", [Bc], F32, kind="ExternalOutput").ap()}
    if dbg:
        outs["dbg_V"] = nc.dram_tensor("dbg_V", [Bc, D_IN], F32,
                                       kind="ExternalOutput").ap()
        for nm in ("dbg_first", "dbg_second", "dbg_deep"):
            outs[nm] = nc.dram_tensor(nm, [Bc], F32, kind="ExternalOutput").ap()
    with tile.TileContext(nc) as tc:
        emit_dfm(tc, outs, ins, Bc=Bc, meta=meta,
                 gather_splits=gather_splits, dbg=dbg, repeat=repeat)
    nc.compile()
    return nc


def run(inputs, trace=False, gather_splits=2, dbg=False, n_cores=None,
        **run_kwargs):
    """Run on 8 cores; returns (y_full, BassKernelResults)."""
    ncores = n_cores or N_CORES
    in_maps, meta = host_prepare(inputs, ncores)
    Bc = in_maps[0]["Xi_big"].shape[0]
    nc = _build_module(in_maps[0], Bc, meta,
                       gather_splits=gather_splits, dbg=dbg)
    res = run_bass_kernel_spmd(
        nc, in_maps, core_ids=list(range(ncores)), trace=trace, **run_kwargs
    )
    y = np.concatenate([r["y"].reshape(-1) for r in res.results])
    return y.astype(np.float32), res


def kernel(**inputs):
    y, _ = run(inputs, trace=False)
    return y
